# revision 12
# baseline (speedup 1.0000x reference)
"""CapsuleLayer (dynamic routing) Trainium2 kernel, SPMD over 8 NeuronCores.

Sharding: input-capsule axis (IN_CAPS=512 -> 64 per core). W and u_hat are
i-sharded; the bij,bijd->bjd contraction is completed with an AllReduce of
s (64x16x32, 131 KB) once per routing iteration.

Per-core layout (i_local = i2*32 + i1, i2 in {0,1}):
  u_hat SBUF [p=(i2*64+b), (i1, j, d)]  -- 128 partitions x 16384 f32
  b/c logits [p, (i1, j)], s/outputs [b, (j, d)].

Phase 1 (per i): u_hat_i[b, jd] = xT_i.T @ W_i on the PE (K=128, M=64, N=512).
Phase 2: routing iterations on the DVE/ACT engines + AllReduce.
"""

import numpy as np

N_CORES = 8
B = 64
IN_CAPS = 512
IN_DIM = 128
N_CAPS = 16
OUT_DIM = 32
I_LOC = IN_CAPS // N_CORES          # 64 input capsules per core
I1 = 32                             # i_local = i2*32 + i1
JD = N_CAPS * OUT_DIM               # 512
EPS = 1e-7
GRP = 4                             # i's per W-DMA/PSUM group
NGRP = I_LOC // GRP                 # 16

# Toggled by test.py for profiling runs.
TRACE = False
TRACE_DIR = None

_cache = {}


def _emit(tc, xT, wT, out, num_routing):
    import concourse.bass as bass
    from concourse import mybir

    from contextlib import ExitStack

    nc = tc.nc
    f32 = mybir.dt.float32
    ctx = ExitStack()
    singles = ctx.enter_context(tc.tile_pool(name="singles", bufs=1))
    wpool = ctx.enter_context(tc.tile_pool(name="wpool", bufs=2))
    pspool = ctx.enter_context(tc.tile_pool(name="pspool", bufs=2, space="PSUM"))
    small = ctx.enter_context(tc.tile_pool(name="small", bufs=2))
    dram = ctx.enter_context(tc.tile_pool(name="dram", bufs=2, space="DRAM"))

    # ---- phase 1: u_hat = einsum over k, per local capsule i ----
    xsb = singles.tile([IN_DIM, I_LOC, B], f32)          # [k, i, b]
    nc.sync.dma_start(xsb[:], xT[:])

    u_hat = singles.tile([128, I1, N_CAPS, OUT_DIM], f32)  # [(i2,b), i1, j, d]

    for g in range(NGRP):
        i2 = (g * GRP) // I1
        i1g = (g * GRP) % I1
        wtile = wpool.tile([IN_DIM, GRP, N_CAPS, OUT_DIM], f32)
        nc.sync.dma_start(
            wtile[:], wT[g * GRP:(g + 1) * GRP].transpose([1, 0, 2, 3])
        )
        # matmul into the partition half matching i2 so the PSUM->SBUF copy
        # stays partition-aligned (engines can't shift partitions)
        ps = pspool.tile([128, GRP, N_CAPS, OUT_DIM], f32)
        for t in range(GRP):
            i = g * GRP + t
            nc.tensor.matmul(
                ps[i2 * B:(i2 + 1) * B, t], xsb[:, i, :], wtile[:, t],
                start=True, stop=True,
            )
        dst = u_hat[i2 * B:(i2 + 1) * B, i1g:i1g + GRP]
        src = ps[i2 * B:(i2 + 1) * B]
        if g % 2 == 0:
            nc.vector.tensor_copy(out=dst, in_=src)
        else:
            nc.scalar.copy(out=dst, in_=src)

    # ---- phase 2: routing ----
    tmp = singles.tile([128, I1, N_CAPS, OUT_DIM], f32)
    b_log = singles.tile([128, I1, N_CAPS], f32)
    out2 = singles.tile([128, N_CAPS, OUT_DIM], f32)
    eps_t = singles.tile([B, 1], f32)
    nc.vector.memset(b_log[:], 0.0)
    nc.vector.memset(eps_t[:], EPS)

    R = num_routing
    for r in range(R):
        if r == 0:
            # b == 0 -> c uniform: s = (1/16) * sum_i u_hat  (scale after AR)
            s_half = small.tile([128, N_CAPS, OUT_DIM], f32)
            nc.vector.reduce_sum(
                out=s_half[:], in_=u_hat.transpose([0, 2, 3, 1]),
                axis=mybir.AxisListType.X,
            )
        else:
            cmax = small.tile([128, I1], f32)
            nc.vector.reduce_max(
                out=cmax[:], in_=b_log[:], axis=mybir.AxisListType.X
            )
            cexp = small.tile([128, I1, N_CAPS], f32)
            nc.vector.tensor_sub(
                cexp[:], b_log[:],
                cmax.unsqueeze(2).broadcast_to([128, I1, N_CAPS]),
            )
            nc.scalar.activation(
                out=cexp[:], in_=cexp[:], func=mybir.ActivationFunctionType.Exp
            )
            csum = small.tile([128, I1], f32)
            nc.vector.reduce_sum(
                out=csum[:], in_=cexp[:], axis=mybir.AxisListType.X
            )
            nc.vector.reciprocal(out=csum[:], in_=csum[:])
            c_t = small.tile([128, I1, N_CAPS], f32)
            nc.vector.tensor_mul(
                c_t[:], cexp[:],
                csum.unsqueeze(2).broadcast_to([128, I1, N_CAPS]),
            )
            nc.vector.tensor_mul(
                tmp[:], u_hat[:],
                c_t.unsqueeze(3).broadcast_to([128, I1, N_CAPS, OUT_DIM]),
            )
            s_half = small.tile([128, N_CAPS, OUT_DIM], f32)
            nc.vector.reduce_sum(
                out=s_half[:], in_=tmp.transpose([0, 2, 3, 1]),
                axis=mybir.AxisListType.X,
            )

        # DVE can't mix base partitions: shift the upper half down via DMA
        s_hi = small.tile([B, N_CAPS, OUT_DIM], f32)
        nc.sync.dma_start(s_hi[:], s_half[B:2 * B])
        s_loc = small.tile([B, N_CAPS, OUT_DIM], f32)
        nc.vector.tensor_add(s_loc[:], s_half[0:B], s_hi[:])

        cc_in = dram.tile([B, N_CAPS, OUT_DIM], f32)
        cc_out = dram.tile([B, N_CAPS, OUT_DIM], f32)
        nc.sync.dma_start(cc_in[:], s_loc[:])
        nc.gpsimd.collective_compute(
            "AllReduce",
            mybir.AluOpType.add,
            replica_groups=[list(range(N_CORES))],
            ins=[cc_in.opt()],
            outs=[cc_out.opt()],
        )
        s_sb = small.tile([B, N_CAPS, OUT_DIM], f32)
        nc.sync.dma_start(s_sb[:], cc_out[:])
        if r == 0:
            nc.vector.tensor_scalar_mul(s_sb[:], s_sb[:], 1.0 / N_CAPS)

        # squash: scale = ss/(1+ss)/sqrt(ss+eps), ss = sum_d s^2
        sq = small.tile([B, N_CAPS, OUT_DIM], f32)
        nc.vector.tensor_mul(sq[:], s_sb[:], s_sb[:])
        ss = small.tile([B, N_CAPS], f32)
        nc.vector.reduce_sum(out=ss[:], in_=sq[:], axis=mybir.AxisListType.X)
        t1 = small.tile([B, N_CAPS], f32)
        nc.scalar.activation(
            out=t1[:], in_=ss[:], func=mybir.ActivationFunctionType.Sqrt,
            bias=eps_t[:], scale=1.0,
        )
        t2 = small.tile([B, N_CAPS], f32)
        nc.vector.tensor_scalar_add(t2[:], ss[:], 1.0)
        nc.vector.tensor_mul(t1[:], t1[:], t2[:])
        nc.vector.reciprocal(out=t1[:], in_=t1[:])
        nc.vector.tensor_mul(t1[:], t1[:], ss[:])   # t1 = squash scale
        out_s = small.tile([B, N_CAPS, OUT_DIM], f32)
        nc.vector.tensor_mul(
            out_s[:], s_sb[:],
            t1.unsqueeze(2).broadcast_to([B, N_CAPS, OUT_DIM]),
        )

        if r == R - 1:
            nc.sync.dma_start(out[:], out_s[:])
        else:
            nc.vector.tensor_copy(out=out2[0:B], in_=out_s[:])
            nc.sync.dma_start(out2[B:2 * B], out_s[:])
            nc.vector.tensor_mul(
                tmp[:], u_hat[:],
                out2.unsqueeze(1).broadcast_to([128, I1, N_CAPS, OUT_DIM]),
            )
            bred = small.tile([128, I1, N_CAPS], f32)
            nc.vector.reduce_sum(
                out=bred[:], in_=tmp[:], axis=mybir.AxisListType.X
            )
            nc.vector.tensor_add(b_log[:], b_log[:], bred[:])

    ctx.close()


def _build(num_routing):
    import concourse.bacc as bacc
    import concourse.tile as tile
    from concourse import mybir

    nc = bacc.Bacc(
        "TRN2", target_bir_lowering=False, debug=False, num_devices=N_CORES,
        dynamic_dma_scratch_size=512,
    )
    f32 = mybir.dt.float32
    xT = nc.dram_tensor("xT", [IN_DIM, I_LOC, B], f32, kind="ExternalInput")
    wT = nc.dram_tensor(
        "wT", [I_LOC, IN_DIM, N_CAPS, OUT_DIM], f32, kind="ExternalInput"
    )
    out = nc.dram_tensor(
        "out", [B, N_CAPS, OUT_DIM], f32, kind="ExternalOutput"
    )
    with tile.TileContext(nc) as tc:
        _emit(tc, xT, wT, out, num_routing)
    nc.compile()
    return nc


def kernel(inputs, W, num_routing):
    from concourse.bass_utils import run_bass_kernel_spmd

    R = int(num_routing)
    assert R >= 1
    if R not in _cache:
        _cache[R] = _build(R)
    nc = _cache[R]

    inputs = np.ascontiguousarray(np.asarray(inputs, dtype=np.float32))
    W = np.asarray(W, dtype=np.float32)

    in_maps = []
    for c in range(N_CORES):
        lo, hi = c * I_LOC, (c + 1) * I_LOC
        xT_c = np.ascontiguousarray(inputs[:, lo:hi, :].transpose(2, 1, 0))
        wT_c = np.ascontiguousarray(W[lo:hi].transpose(0, 2, 1, 3))
        in_maps.append({"xT": xT_c, "wT": wT_c})

    kwargs = {}
    if TRACE:
        kwargs["trace"] = True
        if TRACE_DIR:
            kwargs["tmpdir"] = TRACE_DIR
    res = run_bass_kernel_spmd(
        nc, in_maps, core_ids=list(range(N_CORES)), **kwargs
    )
    if TRACE:
        kernel.last_exec_time_ns = res.exec_time_ns
        kernel.last_results = res
    return np.asarray(res.results[0]["out"], dtype=np.float32)


# revision 20
# speedup vs baseline: 1.1368x; 1.1368x over previous
"""CapsuleLayer (dynamic routing) Trainium2 kernel, SPMD over 8 NeuronCores.

Sharding: input-capsule axis (IN_CAPS=512 -> 64 per core). W and u_hat are
i-sharded; the bij,bijd->bjd contraction is completed with an AllReduce of
s (64x16x32, 131 KB) once per routing iteration.

Per-core layout (i_local = i2*32 + i1, i2 in {0,1}):
  u_hat SBUF [p=(i2*64+b), (i1, j, d)]  -- 128 partitions x 16384 f32
  b/c logits [p, (i1, j)], s/outputs [b, (j, d)].

Phase 1 (per i): u_hat_i[b, jd] = xT_i.T @ W_i on the PE (K=128, M=64, N=512).
Phase 2: routing iterations on the DVE/ACT engines + AllReduce.
"""

import numpy as np

N_CORES = 8
B = 64
IN_CAPS = 512
IN_DIM = 128
N_CAPS = 16
OUT_DIM = 32
I_LOC = IN_CAPS // N_CORES          # 64 input capsules per core
I1 = 32                             # i_local = i2*32 + i1
JD = N_CAPS * OUT_DIM               # 512
EPS = 1e-7
GRP = 4                             # i's per W-DMA/PSUM group
NGRP = I_LOC // GRP                 # 16

# Toggled by test.py for profiling runs.
TRACE = False
TRACE_DIR = None

_cache = {}


def _emit(tc, xT, wT, out, num_routing):
    import concourse.bass as bass
    from concourse import mybir

    from contextlib import ExitStack

    nc = tc.nc
    f32 = mybir.dt.float32
    ctx = ExitStack()
    singles = ctx.enter_context(tc.tile_pool(name="singles", bufs=1))
    wpool = ctx.enter_context(tc.tile_pool(name="wpool", bufs=2))
    pspool = ctx.enter_context(tc.tile_pool(name="pspool", bufs=2, space="PSUM"))
    small = ctx.enter_context(tc.tile_pool(name="small", bufs=2))
    dram = ctx.enter_context(tc.tile_pool(name="dram", bufs=2, space="DRAM"))

    # ---- phase 1: u_hat = einsum over k, per local capsule i ----
    f32r = mybir.dt.float32r
    xsb = singles.tile([IN_DIM, I_LOC, B], f32r)         # [k, i, b]
    nc.sync.dma_start(xsb[:], xT[:])

    u_hat = singles.tile([128, I1, N_CAPS, OUT_DIM], f32)  # [(i2,b), i1, j, d]

    # warm up the collective path while phase 1 streams W (first AllReduce
    # otherwise pays ~18us of one-time ncfw/credit setup on the critical path)
    warm_in = dram.tile([1, 32], f32)
    warm_out = dram.tile([1, 32], f32)
    nc.gpsimd.collective_compute(
        "AllReduce",
        mybir.AluOpType.add,
        replica_groups=[list(range(N_CORES))],
        ins=[warm_in.opt()],
        outs=[warm_out.opt()],
    )

    for g in range(NGRP):
        i2 = (g * GRP) // I1
        i1g = (g * GRP) % I1
        wtile = wpool.tile([IN_DIM, GRP, N_CAPS, OUT_DIM], f32r)
        nc.sync.dma_start(wtile[:], wT[g])
        # matmul into the partition half matching i2 so the PSUM->SBUF copy
        # stays partition-aligned (engines can't shift partitions)
        ps = pspool.tile([128, GRP, N_CAPS, OUT_DIM], f32)
        for t in range(GRP):
            i = g * GRP + t
            lhsT, rhs = xsb[:, i, :], wtile[:, t]
            if i2 == 1:
                # fp32r matmuls may only write PSUM partition base 0
                # (s3d3_mm_valid_dst_partition); run the upper half as fp32
                lhsT, rhs = lhsT.bitcast(f32), rhs.bitcast(f32)
            nc.tensor.matmul(
                ps[i2 * B:(i2 + 1) * B, t], lhsT, rhs,
                start=True, stop=True,
            )
        dst = u_hat[i2 * B:(i2 + 1) * B, i1g:i1g + GRP]
        src = ps[i2 * B:(i2 + 1) * B]
        if g % 2 == 0:
            nc.vector.tensor_copy(out=dst, in_=src)
        else:
            nc.scalar.copy(out=dst, in_=src)

    # ---- phase 2: routing ----
    tmp = singles.tile([128, I1, N_CAPS, OUT_DIM], f32)
    b_log = singles.tile([128, I1, N_CAPS], f32)
    out2 = singles.tile([128, N_CAPS, OUT_DIM], f32)
    eps_t = singles.tile([B, 1], f32)
    nc.vector.memset(b_log[:], 0.0)
    nc.vector.memset(eps_t[:], EPS)

    R = num_routing
    for r in range(R):
        if r == 0:
            # b == 0 -> c uniform: s = (1/16) * sum_i u_hat  (scale after AR)
            nc.vector.tensor_add(
                tmp[:, :I1 // 2], u_hat[:, :I1 // 2], u_hat[:, I1 // 2:]
            )
        else:
            cmax = small.tile([128, I1], f32)
            nc.vector.reduce_max(
                out=cmax[:], in_=b_log[:], axis=mybir.AxisListType.X
            )
            cexp = small.tile([128, I1, N_CAPS], f32)
            nc.vector.tensor_sub(
                cexp[:], b_log[:],
                cmax.unsqueeze(2).broadcast_to([128, I1, N_CAPS]),
            )
            nc.scalar.activation(
                out=cexp[:], in_=cexp[:], func=mybir.ActivationFunctionType.Exp
            )
            csum = small.tile([128, I1], f32)
            nc.vector.reduce_sum(
                out=csum[:], in_=cexp[:], axis=mybir.AxisListType.X
            )
            nc.vector.reciprocal(out=csum[:], in_=csum[:])
            c_t = small.tile([128, I1, N_CAPS], f32)
            nc.vector.tensor_mul(
                c_t[:], cexp[:],
                csum.unsqueeze(2).broadcast_to([128, I1, N_CAPS]),
            )
            nc.vector.tensor_mul(
                tmp[:], u_hat[:],
                c_t.unsqueeze(3).broadcast_to([128, I1, N_CAPS, OUT_DIM]),
            )
            nc.vector.tensor_add(
                tmp[:, :I1 // 2], tmp[:, :I1 // 2], tmp[:, I1 // 2:]
            )
        # in-place contiguous tree over i1 (beats a strided reduce_sum)
        w = I1 // 2
        while w > 1:
            nc.vector.tensor_add(tmp[:, :w // 2], tmp[:, :w // 2], tmp[:, w // 2:w])
            w //= 2
        s_half = tmp[:, 0]  # [128, N_CAPS, OUT_DIM]

        # DVE can't mix base partitions: shift the upper half down via DMA
        s_hi = small.tile([B, N_CAPS, OUT_DIM], f32)
        nc.sync.dma_start(s_hi[:], s_half[B:2 * B])
        s_loc = small.tile([B, N_CAPS, OUT_DIM], f32)
        nc.vector.tensor_add(s_loc[:], s_half[0:B], s_hi[:])

        cc_in = dram.tile([B, N_CAPS, OUT_DIM], f32)
        cc_out = dram.tile([B, N_CAPS, OUT_DIM], f32)
        nc.sync.dma_start(cc_in[:], s_loc[:])
        nc.gpsimd.collective_compute(
            "AllReduce",
            mybir.AluOpType.add,
            replica_groups=[list(range(N_CORES))],
            ins=[cc_in.opt()],
            outs=[cc_out.opt()],
        )
        s_sb = small.tile([B, N_CAPS, OUT_DIM], f32)
        nc.sync.dma_start(s_sb[:], cc_out[:])
        if r == 0:
            nc.vector.tensor_scalar_mul(s_sb[:], s_sb[:], 1.0 / N_CAPS)

        # squash: scale = ss/(1+ss)/sqrt(ss+eps), ss = sum_d s^2
        sq = small.tile([B, N_CAPS, OUT_DIM], f32)
        nc.vector.tensor_mul(sq[:], s_sb[:], s_sb[:])
        ss = small.tile([B, N_CAPS], f32)
        nc.vector.reduce_sum(out=ss[:], in_=sq[:], axis=mybir.AxisListType.X)
        t1 = small.tile([B, N_CAPS], f32)
        nc.scalar.activation(
            out=t1[:], in_=ss[:], func=mybir.ActivationFunctionType.Sqrt,
            bias=eps_t[:], scale=1.0,
        )
        t2 = small.tile([B, N_CAPS], f32)
        nc.vector.tensor_scalar_add(t2[:], ss[:], 1.0)
        nc.vector.tensor_mul(t1[:], t1[:], t2[:])
        nc.vector.reciprocal(out=t1[:], in_=t1[:])
        nc.vector.tensor_mul(t1[:], t1[:], ss[:])   # t1 = squash scale
        out_s = small.tile([B, N_CAPS, OUT_DIM], f32)
        nc.vector.tensor_mul(
            out_s[:], s_sb[:],
            t1.unsqueeze(2).broadcast_to([B, N_CAPS, OUT_DIM]),
        )

        if r == R - 1:
            nc.sync.dma_start(out[:], out_s[:])
        else:
            nc.vector.tensor_copy(out=out2[0:B], in_=out_s[:])
            nc.sync.dma_start(out2[B:2 * B], out_s[:])
            nc.vector.tensor_mul(
                tmp[:], u_hat[:],
                out2.unsqueeze(1).broadcast_to([128, I1, N_CAPS, OUT_DIM]),
            )
            bred = small.tile([128, I1, N_CAPS], f32)
            nc.vector.reduce_sum(
                out=bred[:], in_=tmp[:], axis=mybir.AxisListType.X
            )
            nc.vector.tensor_add(b_log[:], b_log[:], bred[:])

    ctx.close()


def _build(num_routing):
    import concourse.bacc as bacc
    import concourse.tile as tile
    from concourse import mybir

    nc = bacc.Bacc(
        "TRN2", target_bir_lowering=False, debug=False, num_devices=N_CORES,
        dynamic_dma_scratch_size=512,
    )
    f32 = mybir.dt.float32
    f32r = mybir.dt.float32r
    xT = nc.dram_tensor("xT", [IN_DIM, I_LOC, B], f32r, kind="ExternalInput")
    wT = nc.dram_tensor(
        "wT", [NGRP, IN_DIM, GRP, N_CAPS, OUT_DIM], f32r, kind="ExternalInput"
    )
    out = nc.dram_tensor(
        "out", [B, N_CAPS, OUT_DIM], f32, kind="ExternalOutput"
    )
    with tile.TileContext(nc) as tc:
        _emit(tc, xT, wT, out, num_routing)
    nc.compile()
    return nc


def kernel(inputs, W, num_routing):
    from concourse.bass_utils import run_bass_kernel_spmd

    R = int(num_routing)
    assert R >= 1
    if R not in _cache:
        _cache[R] = _build(R)
    nc = _cache[R]

    inputs = np.ascontiguousarray(np.asarray(inputs, dtype=np.float32))
    W = np.asarray(W, dtype=np.float32)

    in_maps = []
    for c in range(N_CORES):
        lo, hi = c * I_LOC, (c + 1) * I_LOC
        xT_c = np.ascontiguousarray(inputs[:, lo:hi, :].transpose(2, 1, 0))
        # [i,j,k,d] -> group-blocked [g, k, t, j, d] so each group DMA is one
        # contiguous 1 MiB block (8 KB per partition row)
        wT_c = np.ascontiguousarray(
            W[lo:hi]
            .reshape(NGRP, GRP, N_CAPS, IN_DIM, OUT_DIM)
            .transpose(0, 3, 1, 2, 4)
        )
        in_maps.append({"xT": xT_c, "wT": wT_c})

    kwargs = {}
    if TRACE:
        kwargs["trace"] = True
        if TRACE_DIR:
            kwargs["tmpdir"] = TRACE_DIR
    res = run_bass_kernel_spmd(
        nc, in_maps, core_ids=list(range(N_CORES)), **kwargs
    )
    if TRACE:
        kernel.last_exec_time_ns = res.exec_time_ns
        kernel.last_results = res
    return np.asarray(res.results[0]["out"], dtype=np.float32)


# revision 21
# speedup vs baseline: 1.5419x; 1.3564x over previous
"""CapsuleLayer (dynamic routing) Trainium2 kernel, SPMD over 8 NeuronCores.

Sharding: input-capsule axis (IN_CAPS=512 -> 64 per core). W and u_hat are
i-sharded; the bij,bijd->bjd contraction is completed with an AllReduce of
s (64x16x32, 131 KB) once per routing iteration.

Per-core layout (i_local = i2*32 + i1, i2 in {0,1}):
  u_hat SBUF [p=(i2*64+b), (d, i1, j)] bf16 -- 128 partitions x 16384
  b/c logits [p, (i1, j)], s/outputs [b, (d, j)].

The (d, i1, j) free order keeps every big DVE pass in the bf16 2x perf mode:
both broadcast multiplies broadcast over a non-innermost dim (innermost stays
step-1), and both reductions are in-place contiguous tree-adds.

Phase 1 (per i): u_hat_i[b, dj] = xT_i.T @ W_i on the PE (K=128, M=64, N=512),
fp32r for the i2=0 half (fp32r may only write PSUM partition base 0), fp32 for
the i2=1 half.
"""

import numpy as np

N_CORES = 8
B = 64
IN_CAPS = 512
IN_DIM = 128
N_CAPS = 16
OUT_DIM = 32
I_LOC = IN_CAPS // N_CORES          # 64 input capsules per core
I1 = 32                             # i_local = i2*32 + i1
JD = N_CAPS * OUT_DIM               # 512
EPS = 1e-7
GRP = 4                             # i's per W-DMA/PSUM group
NGRP = I_LOC // GRP                 # 16

# Toggled by test.py for profiling runs.
TRACE = False
TRACE_DIR = None

_cache = {}


def _emit(tc, xT, wT, out, num_routing):
    from contextlib import ExitStack

    from concourse import mybir

    nc = tc.nc
    f32 = mybir.dt.float32
    f32r = mybir.dt.float32r
    bf16 = mybir.dt.bfloat16
    ctx = ExitStack()
    singles = ctx.enter_context(tc.tile_pool(name="singles", bufs=1))
    wpool = ctx.enter_context(tc.tile_pool(name="wpool", bufs=4))
    pspool = ctx.enter_context(tc.tile_pool(name="pspool", bufs=2, space="PSUM"))
    small = ctx.enter_context(tc.tile_pool(name="small", bufs=2))
    dram = ctx.enter_context(tc.tile_pool(name="dram", bufs=2, space="DRAM"))

    # ---- phase 1: u_hat = einsum over k, per local capsule i ----
    xsb = singles.tile([IN_DIM, I_LOC, B], f32r)         # [k, i, b]
    nc.sync.dma_start(xsb[:], xT[:])

    u_hat = singles.tile([128, OUT_DIM, I1, N_CAPS], bf16)  # [(i2,b), d, i1, j]

    # warm up the collective path while phase 1 streams W: the first AllReduce
    # of a given size otherwise pays ~20us of one-time setup on the critical
    # path. Same shape as the real ones.
    warm_in = dram.tile([B, OUT_DIM, N_CAPS], f32)
    warm_out = dram.tile([B, OUT_DIM, N_CAPS], f32)
    nc.gpsimd.collective_compute(
        "AllReduce",
        mybir.AluOpType.add,
        replica_groups=[list(range(N_CORES))],
        ins=[warm_in.opt()],
        outs=[warm_out.opt()],
    )

    for g in range(NGRP):
        i2 = (g * GRP) // I1
        i1g = (g * GRP) % I1
        wtile = wpool.tile([IN_DIM, GRP, OUT_DIM, N_CAPS], f32r)
        nc.sync.dma_start(wtile[:], wT[g])
        # fp32r matmuls may only write PSUM partition base 0
        # (s3d3_mm_valid_dst_partition); run the upper half as fp32.
        ps = pspool.tile([128, GRP, OUT_DIM, N_CAPS], f32)
        for t in range(GRP):
            i = g * GRP + t
            lhsT, rhs = xsb[:, i, :], wtile[:, t]
            if i2 == 1:
                lhsT, rhs = lhsT.bitcast(f32), rhs.bitcast(f32)
            nc.tensor.matmul(
                ps[i2 * B:(i2 + 1) * B, t], lhsT, rhs,
                start=True, stop=True,
            )
        # copy+cast PSUM f32 -> SBUF bf16; dst viewed (i1, d, j) to match src
        dst = u_hat[i2 * B:(i2 + 1) * B, :, i1g:i1g + GRP, :].transpose(
            [0, 2, 1, 3]
        )
        src = ps[i2 * B:(i2 + 1) * B]
        if g % 2 == 0:
            nc.vector.tensor_copy(out=dst, in_=src)
        else:
            nc.scalar.copy(out=dst, in_=src)

    # ---- phase 2: routing ----
    tmp = singles.tile([128, OUT_DIM, I1, N_CAPS], bf16)
    b_log = singles.tile([128, I1, N_CAPS], f32)
    out2 = singles.tile([128, OUT_DIM, N_CAPS], bf16)
    eps_t = singles.tile([B, 1], f32)
    nc.vector.memset(b_log[:], 0.0)
    nc.vector.memset(eps_t[:], EPS)

    R = num_routing
    for r in range(R):
        if r == 0:
            # b == 0 -> c uniform: s = (1/16) * sum_i u_hat (scale after AR)
            nc.vector.tensor_add(
                tmp[:, :, :I1 // 2], u_hat[:, :, :I1 // 2],
                u_hat[:, :, I1 // 2:],
            )
        else:
            cmax = small.tile([128, I1], f32)
            nc.vector.reduce_max(
                out=cmax[:], in_=b_log[:], axis=mybir.AxisListType.X
            )
            cexp = small.tile([128, I1, N_CAPS], f32)
            nc.vector.tensor_sub(
                cexp[:], b_log[:],
                cmax.unsqueeze(2).broadcast_to([128, I1, N_CAPS]),
            )
            nc.scalar.activation(
                out=cexp[:], in_=cexp[:], func=mybir.ActivationFunctionType.Exp
            )
            csum = small.tile([128, I1], f32)
            nc.vector.reduce_sum(
                out=csum[:], in_=cexp[:], axis=mybir.AxisListType.X
            )
            nc.vector.reciprocal(out=csum[:], in_=csum[:])
            c_t = small.tile([128, I1, N_CAPS], bf16)
            nc.vector.tensor_mul(
                c_t[:], cexp[:],
                csum.unsqueeze(2).broadcast_to([128, I1, N_CAPS]),
            )
            # s-mul: broadcast c over outermost d keeps bf16 2x mode
            nc.vector.tensor_mul(
                tmp[:], u_hat[:],
                c_t.unsqueeze(1).broadcast_to([128, OUT_DIM, I1, N_CAPS]),
            )
            nc.vector.tensor_add(
                tmp[:, :, :I1 // 2], tmp[:, :, :I1 // 2], tmp[:, :, I1 // 2:]
            )
        # contiguous in-place tree over i1 (middle dim); final level -> f32
        w = I1 // 2
        while w > 2:
            nc.vector.tensor_add(
                tmp[:, :, :w // 2], tmp[:, :, :w // 2], tmp[:, :, w // 2:w]
            )
            w //= 2
        s_half = small.tile([128, OUT_DIM, N_CAPS], f32)
        nc.vector.tensor_add(s_half[:], tmp[:, :, 0, :], tmp[:, :, 1, :])

        # DVE can't mix base partitions: shift the upper half down via DMA
        s_hi = small.tile([B, OUT_DIM, N_CAPS], f32)
        nc.sync.dma_start(s_hi[:], s_half[B:2 * B])
        s_loc = small.tile([B, OUT_DIM, N_CAPS], f32)
        nc.vector.tensor_add(s_loc[:], s_half[0:B], s_hi[:])

        cc_in = dram.tile([B, OUT_DIM, N_CAPS], f32)
        cc_out = dram.tile([B, OUT_DIM, N_CAPS], f32)
        nc.sync.dma_start(cc_in[:], s_loc[:])
        nc.gpsimd.collective_compute(
            "AllReduce",
            mybir.AluOpType.add,
            replica_groups=[list(range(N_CORES))],
            ins=[cc_in.opt()],
            outs=[cc_out.opt()],
        )
        s_sb = small.tile([B, OUT_DIM, N_CAPS], f32)
        nc.sync.dma_start(s_sb[:], cc_out[:])
        if r == 0:
            nc.vector.tensor_scalar_mul(s_sb[:], s_sb[:], 1.0 / N_CAPS)

        # squash: scale = ss/(1+ss)/sqrt(ss+eps), ss = sum_d s^2
        sq = small.tile([B, OUT_DIM, N_CAPS], f32)
        nc.vector.tensor_mul(sq[:], s_sb[:], s_sb[:])
        ss = small.tile([B, N_CAPS], f32)
        nc.vector.reduce_sum(
            out=ss[:], in_=sq.transpose([0, 2, 1]), axis=mybir.AxisListType.X
        )
        t1 = small.tile([B, N_CAPS], f32)
        nc.scalar.activation(
            out=t1[:], in_=ss[:], func=mybir.ActivationFunctionType.Sqrt,
            bias=eps_t[:], scale=1.0,
        )
        t2 = small.tile([B, N_CAPS], f32)
        nc.vector.tensor_scalar_add(t2[:], ss[:], 1.0)
        nc.vector.tensor_mul(t1[:], t1[:], t2[:])
        nc.vector.reciprocal(out=t1[:], in_=t1[:])
        nc.vector.tensor_mul(t1[:], t1[:], ss[:])   # t1 = squash scale
        out_s = small.tile([B, OUT_DIM, N_CAPS], f32)
        nc.vector.tensor_mul(
            out_s[:], s_sb[:],
            t1.unsqueeze(1).broadcast_to([B, OUT_DIM, N_CAPS]),
        )

        if r == R - 1:
            out_t = small.tile([B, N_CAPS, OUT_DIM], f32)
            nc.vector.tensor_copy(out=out_t[:], in_=out_s.transpose([0, 2, 1]))
            nc.sync.dma_start(out[:], out_t[:])
        else:
            nc.vector.tensor_copy(out=out2[0:B], in_=out_s[:])
            nc.sync.dma_start(out2[B:2 * B], out2[0:B])
            # bu-mul: broadcast outputs over middle i1 keeps bf16 2x mode
            nc.vector.tensor_mul(
                tmp[:], u_hat[:],
                out2.unsqueeze(2).broadcast_to([128, OUT_DIM, I1, N_CAPS]),
            )
            w = OUT_DIM
            while w > 2:
                nc.vector.tensor_add(
                    tmp[:, :w // 2], tmp[:, :w // 2], tmp[:, w // 2:w]
                )
                w //= 2
            bred = small.tile([128, I1, N_CAPS], f32)
            nc.vector.tensor_add(bred[:], tmp[:, 0], tmp[:, 1])
            nc.vector.tensor_add(b_log[:], b_log[:], bred[:])

    ctx.close()


def _build(num_routing):
    import concourse.bacc as bacc
    import concourse.tile as tile
    from concourse import mybir

    nc = bacc.Bacc(
        "TRN2", target_bir_lowering=False, debug=False, num_devices=N_CORES,
        dynamic_dma_scratch_size=512,
    )
    f32 = mybir.dt.float32
    f32r = mybir.dt.float32r
    xT = nc.dram_tensor("xT", [IN_DIM, I_LOC, B], f32r, kind="ExternalInput")
    wT = nc.dram_tensor(
        "wT", [NGRP, IN_DIM, GRP, OUT_DIM, N_CAPS], f32r, kind="ExternalInput"
    )
    out = nc.dram_tensor(
        "out", [B, N_CAPS, OUT_DIM], f32, kind="ExternalOutput"
    )
    with tile.TileContext(nc) as tc:
        _emit(tc, xT, wT, out, num_routing)
    nc.compile()
    return nc


def kernel(inputs, W, num_routing):
    from concourse.bass_utils import run_bass_kernel_spmd

    R = int(num_routing)
    assert R >= 1
    if R not in _cache:
        _cache[R] = _build(R)
    nc = _cache[R]

    inputs = np.ascontiguousarray(np.asarray(inputs, dtype=np.float32))
    W = np.asarray(W, dtype=np.float32)

    in_maps = []
    for c in range(N_CORES):
        lo, hi = c * I_LOC, (c + 1) * I_LOC
        xT_c = np.ascontiguousarray(inputs[:, lo:hi, :].transpose(2, 1, 0))
        # [i,j,k,d] -> group-blocked [g, k, t, d, j] so each group DMA is one
        # contiguous 1 MiB block and PSUM columns come out in (d, j) order
        wT_c = np.ascontiguousarray(
            W[lo:hi]
            .reshape(NGRP, GRP, N_CAPS, IN_DIM, OUT_DIM)
            .transpose(0, 3, 1, 4, 2)
        )
        in_maps.append({"xT": xT_c, "wT": wT_c})

    kwargs = {}
    if TRACE:
        kwargs["trace"] = True
        if TRACE_DIR:
            kwargs["tmpdir"] = TRACE_DIR
    res = run_bass_kernel_spmd(
        nc, in_maps, core_ids=list(range(N_CORES)), **kwargs
    )
    if TRACE:
        kernel.last_exec_time_ns = res.exec_time_ns
        kernel.last_results = res
    return np.asarray(res.results[0]["out"], dtype=np.float32)


# revision 26
# speedup vs baseline: 1.6484x; 1.0691x over previous
"""CapsuleLayer (dynamic routing) Trainium2 kernel, SPMD over 8 NeuronCores.

Sharding: input-capsule axis (IN_CAPS=512 -> 64 per core). W and u_hat are
i-sharded; the bij,bijd->bjd contraction is completed with an AllReduce of
s (64x16x32, 131 KB) once per routing iteration.

Per-core layout (i_local = i2*32 + i1, i2 in {0,1}):
  u_hat SBUF [p=(i2*64+b), (d, i1, j)] bf16 -- 128 partitions x 16384
  b/c logits [p, (i1, j)], s/outputs [b, (d, j)].

The (d, i1, j) free order keeps every big DVE pass in the bf16 2x perf mode:
both broadcast multiplies broadcast over a non-innermost dim (innermost stays
step-1), and both reductions are in-place contiguous tree-adds.

Phase 1 (per i): u_hat_i[b, dj] = xT_i.T @ W_i on the PE (K=128, M=64, N=512),
fp32r for the i2=0 half (fp32r may only write PSUM partition base 0), fp32 for
the i2=1 half.
"""

import numpy as np

N_CORES = 8
B = 64
IN_CAPS = 512
IN_DIM = 128
N_CAPS = 16
OUT_DIM = 32
I_LOC = IN_CAPS // N_CORES          # 64 input capsules per core
I1 = 32                             # i_local = i2*32 + i1
JD = N_CAPS * OUT_DIM               # 512
EPS = 1e-7
GRP = 4                             # i's per W-DMA/PSUM group
NGRP = I_LOC // GRP                 # 16

# Toggled by test.py for profiling runs.
TRACE = False
TRACE_DIR = None

_cache = {}


def _emit(tc, xT, wT, out, num_routing):
    from contextlib import ExitStack

    from concourse import mybir

    nc = tc.nc
    f32 = mybir.dt.float32
    f32r = mybir.dt.float32r
    bf16 = mybir.dt.bfloat16
    ctx = ExitStack()
    singles = ctx.enter_context(tc.tile_pool(name="singles", bufs=1))
    wpool = ctx.enter_context(tc.tile_pool(name="wpool", bufs=4))
    pspool = ctx.enter_context(tc.tile_pool(name="pspool", bufs=2, space="PSUM"))
    small = ctx.enter_context(tc.tile_pool(name="small", bufs=2))
    dram = ctx.enter_context(tc.tile_pool(name="dram", bufs=2, space="DRAM"))

    # warm up the collective path while phase 1 streams W: the first couple of
    # AllReduces otherwise pay ~20-30us of one-time setup on the critical
    # path. Same shape as the real ones.
    for _ in range(2):
        warm_in = dram.tile([B, OUT_DIM, N_CAPS], f32, name=f"warm_in{_}")
        warm_out = dram.tile([B, OUT_DIM, N_CAPS], f32, name=f"warm_out{_}")
        nc.gpsimd.collective_compute(
            "AllReduce",
            mybir.AluOpType.add,
            replica_groups=[list(range(N_CORES))],
            ins=[warm_in.opt()],
            outs=[warm_out.opt()],
        )

    # ---- phase 1: u_hat = einsum over k, per local capsule i ----
    xsb = singles.tile([IN_DIM, I_LOC, B], f32r)         # [k, i, b]
    for q in range(4):
        nc.sync.dma_start(
            xsb[:, q * (I_LOC // 4):(q + 1) * (I_LOC // 4), :],
            xT[:, q * (I_LOC // 4):(q + 1) * (I_LOC // 4), :],
        )

    u_hat = singles.tile([128, OUT_DIM, I1, N_CAPS], bf16)  # [(i2,b), d, i1, j]
    # s0 accumulator: c is uniform in iteration 0, so sum_i u_hat can be
    # accumulated group-by-group during phase 1 while the DVE is mostly idle
    s_acc = singles.tile([128, OUT_DIM, N_CAPS], f32)
    nc.vector.memset(s_acc[:], 0.0)

    for g in range(NGRP):
        i2 = (g * GRP) // I1
        i1g = (g * GRP) % I1
        wtile = wpool.tile([IN_DIM, GRP, OUT_DIM, N_CAPS], f32r)
        nc.sync.dma_start(wtile[:], wT[g])
        # fp32r matmuls may only write PSUM partition base 0
        # (s3d3_mm_valid_dst_partition); run the upper half as fp32.
        ps = pspool.tile([128, GRP, OUT_DIM, N_CAPS], f32)
        for t in range(GRP):
            i = g * GRP + t
            lhsT, rhs = xsb[:, i, :], wtile[:, t]
            if i2 == 1:
                lhsT, rhs = lhsT.bitcast(f32), rhs.bitcast(f32)
            nc.tensor.matmul(
                ps[i2 * B:(i2 + 1) * B, t], lhsT, rhs,
                start=True, stop=True,
            )
        # copy+cast PSUM f32 -> SBUF bf16; dst viewed (i1, d, j) to match src
        dst = u_hat[i2 * B:(i2 + 1) * B, :, i1g:i1g + GRP, :].transpose(
            [0, 2, 1, 3]
        )
        src = ps[i2 * B:(i2 + 1) * B]
        if g % 2 == 0:
            nc.vector.tensor_copy(out=dst, in_=src)
        else:
            nc.scalar.copy(out=dst, in_=src)
        # fold this group's 4 capsules into the iteration-0 s accumulator
        lo, hi = i2 * B, (i2 + 1) * B
        ga = small.tile([128, OUT_DIM, N_CAPS], bf16, name="ga")
        gb = small.tile([128, OUT_DIM, N_CAPS], bf16, name="gb")
        nc.vector.tensor_add(
            ga[lo:hi], u_hat[lo:hi, :, i1g, :], u_hat[lo:hi, :, i1g + 1, :]
        )
        nc.vector.tensor_add(
            gb[lo:hi], u_hat[lo:hi, :, i1g + 2, :], u_hat[lo:hi, :, i1g + 3, :]
        )
        nc.vector.tensor_add(ga[lo:hi], ga[lo:hi], gb[lo:hi])
        nc.vector.tensor_add(s_acc[lo:hi], s_acc[lo:hi], ga[lo:hi])

    # ---- phase 2: routing ----
    tmp = singles.tile([128, OUT_DIM, I1, N_CAPS], bf16)
    b_log = singles.tile([128, I1, N_CAPS], f32)
    out2 = singles.tile([128, OUT_DIM, N_CAPS], bf16)
    eps_t = singles.tile([B, 1], f32)
    nc.vector.memset(b_log[:], 0.0)
    nc.vector.memset(eps_t[:], EPS)

    R = num_routing
    for r in range(R):
        if r == 0:
            # s0 already accumulated into s_acc during phase 1
            pass
        else:
            cmax = small.tile([128, I1], f32)
            nc.vector.reduce_max(
                out=cmax[:], in_=b_log[:], axis=mybir.AxisListType.X
            )
            cexp = small.tile([128, I1, N_CAPS], f32)
            nc.vector.tensor_sub(
                cexp[:], b_log[:],
                cmax.unsqueeze(2).broadcast_to([128, I1, N_CAPS]),
            )
            nc.scalar.activation(
                out=cexp[:], in_=cexp[:], func=mybir.ActivationFunctionType.Exp
            )
            csum = small.tile([128, I1], f32)
            nc.vector.reduce_sum(
                out=csum[:], in_=cexp[:], axis=mybir.AxisListType.X
            )
            nc.vector.reciprocal(out=csum[:], in_=csum[:])
            c_t = small.tile([128, I1, N_CAPS], bf16)
            nc.vector.tensor_mul(
                c_t[:], cexp[:],
                csum.unsqueeze(2).broadcast_to([128, I1, N_CAPS]),
            )
            # s-mul: broadcast c over outermost d keeps bf16 2x mode
            nc.vector.tensor_mul(
                tmp[:], u_hat[:],
                c_t.unsqueeze(1).broadcast_to([128, OUT_DIM, I1, N_CAPS]),
            )
            nc.vector.tensor_add(
                tmp[:, :, :I1 // 2], tmp[:, :, :I1 // 2], tmp[:, :, I1 // 2:]
            )
        if r == 0:
            s_half = s_acc
        else:
            # contiguous in-place tree over i1 (middle); final level -> f32
            w = I1 // 2
            while w > 2:
                nc.vector.tensor_add(
                    tmp[:, :, :w // 2], tmp[:, :, :w // 2], tmp[:, :, w // 2:w]
                )
                w //= 2
            s_half = small.tile([128, OUT_DIM, N_CAPS], f32)
            nc.vector.tensor_add(s_half[:], tmp[:, :, 0, :], tmp[:, :, 1, :])

        # DVE can't mix base partitions: shift the upper half down via DMA
        s_hi = small.tile([B, OUT_DIM, N_CAPS], f32)
        nc.sync.dma_start(s_hi[:], s_half[B:2 * B])
        s_loc = small.tile([B, OUT_DIM, N_CAPS], f32)
        nc.vector.tensor_add(s_loc[:], s_half[0:B], s_hi[:])

        cc_in = dram.tile([B, OUT_DIM, N_CAPS], f32)
        cc_out = dram.tile([B, OUT_DIM, N_CAPS], f32)
        nc.sync.dma_start(cc_in[:], s_loc[:])
        nc.gpsimd.collective_compute(
            "AllReduce",
            mybir.AluOpType.add,
            replica_groups=[list(range(N_CORES))],
            ins=[cc_in.opt()],
            outs=[cc_out.opt()],
        )
        s_sb = small.tile([B, OUT_DIM, N_CAPS], f32)
        nc.sync.dma_start(s_sb[:], cc_out[:])
        if r == 0:
            nc.vector.tensor_scalar_mul(s_sb[:], s_sb[:], 1.0 / N_CAPS)

        # squash: scale = ss/(1+ss)/sqrt(ss+eps), ss = sum_d s^2
        sq = small.tile([B, OUT_DIM, N_CAPS], f32)
        nc.vector.tensor_mul(sq[:], s_sb[:], s_sb[:])
        ss = small.tile([B, N_CAPS], f32)
        nc.vector.reduce_sum(
            out=ss[:], in_=sq.transpose([0, 2, 1]), axis=mybir.AxisListType.X
        )
        t1 = small.tile([B, N_CAPS], f32)
        nc.scalar.activation(
            out=t1[:], in_=ss[:], func=mybir.ActivationFunctionType.Sqrt,
            bias=eps_t[:], scale=1.0,
        )
        t2 = small.tile([B, N_CAPS], f32)
        nc.vector.tensor_scalar_add(t2[:], ss[:], 1.0)
        nc.vector.tensor_mul(t1[:], t1[:], t2[:])
        nc.vector.reciprocal(out=t1[:], in_=t1[:])
        nc.vector.tensor_mul(t1[:], t1[:], ss[:])   # t1 = squash scale
        out_s = small.tile([B, OUT_DIM, N_CAPS], f32)
        nc.vector.tensor_mul(
            out_s[:], s_sb[:],
            t1.unsqueeze(1).broadcast_to([B, OUT_DIM, N_CAPS]),
        )

        if r == R - 1:
            out_t = small.tile([B, N_CAPS, OUT_DIM], f32)
            nc.vector.tensor_copy(out=out_t[:], in_=out_s.transpose([0, 2, 1]))
            nc.sync.dma_start(out[:], out_t[:])
        else:
            nc.vector.tensor_copy(out=out2[0:B], in_=out_s[:])
            nc.sync.dma_start(out2[B:2 * B], out2[0:B])
            # bu-mul: broadcast outputs over middle i1 keeps bf16 2x mode
            nc.vector.tensor_mul(
                tmp[:], u_hat[:],
                out2.unsqueeze(2).broadcast_to([128, OUT_DIM, I1, N_CAPS]),
            )
            w = OUT_DIM
            while w > 2:
                nc.vector.tensor_add(
                    tmp[:, :w // 2], tmp[:, :w // 2], tmp[:, w // 2:w]
                )
                w //= 2
            bred = small.tile([128, I1, N_CAPS], f32)
            nc.vector.tensor_add(bred[:], tmp[:, 0], tmp[:, 1])
            nc.vector.tensor_add(b_log[:], b_log[:], bred[:])

    ctx.close()


def _build(num_routing):
    import concourse.bacc as bacc
    import concourse.tile as tile
    from concourse import mybir

    nc = bacc.Bacc(
        "TRN2", target_bir_lowering=False, debug=False, num_devices=N_CORES,
        dynamic_dma_scratch_size=512,
    )
    f32 = mybir.dt.float32
    f32r = mybir.dt.float32r
    xT = nc.dram_tensor("xT", [IN_DIM, I_LOC, B], f32r, kind="ExternalInput")
    wT = nc.dram_tensor(
        "wT", [NGRP, IN_DIM, GRP, OUT_DIM, N_CAPS], f32r, kind="ExternalInput"
    )
    out = nc.dram_tensor(
        "out", [B, N_CAPS, OUT_DIM], f32, kind="ExternalOutput"
    )
    with tile.TileContext(nc) as tc:
        _emit(tc, xT, wT, out, num_routing)
    nc.compile()
    return nc


def kernel(inputs, W, num_routing):
    from concourse.bass_utils import run_bass_kernel_spmd

    R = int(num_routing)
    assert R >= 1
    if R not in _cache:
        _cache[R] = _build(R)
    nc = _cache[R]

    inputs = np.ascontiguousarray(np.asarray(inputs, dtype=np.float32))
    W = np.asarray(W, dtype=np.float32)

    in_maps = []
    for c in range(N_CORES):
        lo, hi = c * I_LOC, (c + 1) * I_LOC
        xT_c = np.ascontiguousarray(inputs[:, lo:hi, :].transpose(2, 1, 0))
        # [i,j,k,d] -> group-blocked [g, k, t, d, j] so each group DMA is one
        # contiguous 1 MiB block and PSUM columns come out in (d, j) order
        wT_c = np.ascontiguousarray(
            W[lo:hi]
            .reshape(NGRP, GRP, N_CAPS, IN_DIM, OUT_DIM)
            .transpose(0, 3, 1, 4, 2)
        )
        in_maps.append({"xT": xT_c, "wT": wT_c})

    kwargs = {}
    if TRACE:
        kwargs["trace"] = True
        if TRACE_DIR:
            kwargs["tmpdir"] = TRACE_DIR
    res = run_bass_kernel_spmd(
        nc, in_maps, core_ids=list(range(N_CORES)), **kwargs
    )
    if TRACE:
        kernel.last_exec_time_ns = res.exec_time_ns
        kernel.last_results = res
    return np.asarray(res.results[0]["out"], dtype=np.float32)
